# revision 1
# baseline (speedup 1.0000x reference)
"""Trainium2 Bass kernel for GQA attention with ALiBi + sliding window + QK-RMSNorm.

Sharding: tensor-parallel over heads across 8 cores. Core c owns q-heads
[4c,4c+4) and kv-head c. Each core computes a partial output through its
wo column-shard; host sums the 8 partials. The RMSNorm over the full
(flattened-heads) axis needs a cross-core sum-of-squares; one tiny
AllReduce per batch, pipelined behind the other batch's projections.

v3 design:
- Projections/wo weight-stationary in bf16 (N=512 moving, fp32 PSUM).
- Q stored qpair-packed: qq[66, 16*1024], cols = qpair-major, then
  4 heads x 256 queries. Each score matmul streams N=512 (2 heads) per
  key-block stationary -> half the matmul+ldweights count vs per-head
  N=256.
- ALiBi folded into the score matmul via augmented contraction rows:
  k_hat=[k; s~; 1], qq rows 64/65 = [slope_h; -slope_h*t~] (positions
  shifted by -1024; the shift cancels in s~-t~). Causal/window edges are
  masked with DVE adds of constant -1e30 tiles on <=4 blocks per qpair.
- exp on scalar -> bf16 P; PV: v_hat [128,65] bf16 stationary, ones col
  = softmax denominator; divide via reciprocal_approx_fast + rank-1
  broadcast. GpSimd takes the copy traffic (otherwise idle).
"""
import sys, os
sys.path.insert(0, "/opt/trn_rl_repo")

import numpy as np

B, T, DIM = 2, 2048, 2048
NH, NKV, HD = 32, 8, 64
WINDOW = 1024
EPS = 1e-6
T4 = B * T            # 4096 flattened tokens
QH = NH // 8          # 4 q heads per core
QD = QH * HD          # 256 q dims per core
TC = 512              # projection token chunk
NCH = T4 // TC        # 8 chunks (4 per batch)
TQ = 256              # attention query tile (pair of 128-blocks)
NKT = DIM // 128      # 16 k-tiles for projections

_CACHE = {}


def _build_bass():
    from concourse import bass, bacc, mybir
    from concourse.tile import TileContext

    dt = mybir.dt.float32
    dtr = mybir.dt.float32r
    bf = mybir.dt.bfloat16
    AF = mybir.ActivationFunctionType

    nc = bacc.Bacc("TRN2", target_bir_lowering=False, debug=False,
                   num_devices=8)

    xT = nc.dram_tensor("xT", [DIM, T4], bf, kind="ExternalInput")
    wT = nc.dram_tensor("wT", [DIM, QD + 2 * HD], bf, kind="ExternalInput")
    woT = nc.dram_tensor("woT", [QD, DIM], bf, kind="ExternalInput")
    qnw = nc.dram_tensor("qnw", [1, QD], dtr, kind="ExternalInput")
    knw = nc.dram_tensor("knw", [1, HD], dtr, kind="ExternalInput")
    aug = nc.dram_tensor("aug", [2, T4], dtr, kind="ExternalInput")
    aug2 = nc.dram_tensor("aug2", [2, 4 * T4], dtr, kind="ExternalInput")
    masks = nc.dram_tensor("masks", [4, 128, 2 * TQ], dt, kind="ExternalInput")
    onesc_in = nc.dram_tensor("ones_col", [128, 1], dtr, kind="ExternalInput")
    onesr_in = nc.dram_tensor("ones_row", [1, 64], dtr, kind="ExternalInput")
    ident_in = nc.dram_tensor("ident", [64, 64], dt, kind="ExternalInput")
    sc_in = nc.dram_tensor("sc_col", [128, 32], dt, kind="ExternalInput")
    bi_in = nc.dram_tensor("bi_col", [128, 32], dt, kind="ExternalInput")
    out_d = nc.dram_tensor("out", [DIM, T4], bf, kind="ExternalOutput")

    # alternate copies between Vector and Scalar (GpSimd cannot touch PSUM)
    cp_i = [0]

    def rr_copy(dst, src):
        cp_i[0] += 1
        if cp_i[0] % 2:
            nc.vector.tensor_copy(dst, src)
        else:
            nc.scalar.copy(dst, src)

    with TileContext(nc) as tc:
        with (
            tc.tile_pool(name="consts", bufs=1) as cp,
            tc.tile_pool(name="persist", bufs=1) as pp,
            tc.tile_pool(name="xin", bufs=2) as xp,
            tc.tile_pool(name="work", bufs=2) as wk,
            tc.tile_pool(name="expp", bufs=4) as ep,
            tc.tile_pool(name="outp", bufs=3) as op_,
            tc.tile_pool(name="dram", bufs=1, space="DRAM") as dp,
        ):
            # ---- constants / weights, loaded once ----
            wtiles = []
            for kt in range(NKT):
                t = cp.tile([128, QD + 2 * HD], bf, tag=f"w{kt}")
                nc.sync.dma_start(t[:], wT[kt * 128:(kt + 1) * 128, :])
                wtiles.append(t)
            wo_sb = []
            for p in range(2):
                t = cp.tile([128, DIM], bf, tag=f"wo{p}")
                nc.sync.dma_start(t[:], woT[p * 128:(p + 1) * 128, :])
                wo_sb.append(t)
            mask_sb = []
            for m in range(4):
                t = cp.tile([128, 2 * TQ], dt, tag=f"m{m}")
                nc.sync.dma_start(t[:], masks[m])
                mask_sb.append(t)
            qnw_sb = cp.tile([1, QD], dtr, tag="qnw")
            nc.sync.dma_start(qnw_sb[:], qnw[:])
            knw_sb = cp.tile([1, HD], dtr, tag="knw")
            nc.sync.dma_start(knw_sb[:], knw[:])
            sc_col = cp.tile([128, 32], dt, tag="sc")
            nc.sync.dma_start(sc_col[:], sc_in[:])
            bi_col = cp.tile([128, 32], dt, tag="bi")
            nc.sync.dma_start(bi_col[:], bi_in[:])
            ones_col = cp.tile([128, 1], dtr, tag="oc")
            nc.sync.dma_start(ones_col[:], onesc_in[:])
            ones_row = cp.tile([1, 64], dtr, tag="or")
            nc.sync.dma_start(ones_row[:], onesr_in[:])
            ident = cp.tile([64, 64], dt, tag="id")
            nc.sync.dma_start(ident[:], ident_in[:])

            # ---- persistent activations ----
            # qq: qpair-packed q. col = gq*1024 + h*256 + ql for global
            # qpair gq (= token//256), local head h, local query ql.
            # rows 0:64 = q dims, row 64 = slope_h, row 65 = -slope_h*t~
            qq = pp.tile([66, 4 * T4], dtr, tag="qq", name="qq")
            nc.sync.dma_start(qq[64:65, :], aug2[0:1, :])
            nc.sync.dma_start(qq[65:66, :], aug2[1:2, :])
            # k_hat: rows 0:64 = k dims, row 64 = s~, row 65 = 1
            kh = pp.tile([66, T4], dtr, tag="kT", name="kT")
            nc.sync.dma_start(kh[64:65, :], aug[0:1, :])
            nc.sync.dma_start(kh[65:66, :], aug[1:2, :])
            # v_hat per 128-token key block: [128, 64+1] bf16, col 64 = 1
            vaug = []
            for sb in range(T4 // 128):
                t = pp.tile([128, HD + 1], bf, tag=f"v{sb}")
                nc.vector.memset(t[:, HD:HD + 1], 1.0)
                vaug.append(t)
            # attention output (divided), per head-pair: rows 0:64 head 2p,
            # rows 64:128 head 2p+1
            o2 = [pp.tile([128, T4], bf, tag=f"o{p}", name=f"o{p}")
                  for p in range(2)]

            cc_in = [dp.tile([2, T], dt, name=f"cci{b}") for b in range(B)]
            cc_out = [dp.tile([2, T], dt, name=f"cco{b}") for b in range(B)]
            rs_dram = [dp.tile([2, T], dtr, name=f"rsd{b}") for b in range(B)]

            # ============ phase 1: projections + sumsq (per batch) ============
            from concourse import mybir as _mb

            with (
                tc.tile_pool(name="ps_proj", bufs=3, space="PSUM") as pj,
                tc.tile_pool(name="ps_ss", bufs=2, space="PSUM") as pss,
                tc.tile_pool(name="ps_tr", bufs=1, space="PSUM") as ptr,
            ):
                for b in range(B):
                    for bc in range(NCH // B):
                        ch = b * (NCH // B) + bc
                        ts0 = ch * TC
                        tsl = slice(ts0, ts0 + TC)
                        xts = []
                        for kt in range(NKT):
                            t = xp.tile([128, TC], bf, tag=f"x{kt}")
                            nc.sync.dma_start(
                                t[:], xT[kt * 128:(kt + 1) * 128, tsl])
                            xts.append(t)
                        sspsum = pss.tile([1, TC], dt, tag="ss")
                        sskp = pss.tile([1, TC], dt, tag="ssk_ps")
                        for mt in range(3):
                            ppsum = pj.tile([128, TC], dt, tag="pj")
                            for kt in range(NKT):
                                nc.tensor.matmul(
                                    ppsum[:],
                                    wtiles[kt][:, mt * 128:(mt + 1) * 128],
                                    xts[kt][:],
                                    start=(kt == 0), stop=(kt == NKT - 1))
                            if mt < 2:
                                # copy into qq layout + sumsq; the squares
                                # tile spans both qpairs so each ss matmul
                                # covers the full [1,512] row (one clean
                                # accumulation group of 4 on the ss bank)
                                for hh in range(2):
                                    h = 2 * mt + hh
                                    sq = wk.tile([64, TC], dtr,
                                                 tag="sq", name="sq")
                                    for qp in range(2):
                                        gq = 2 * ch + qp
                                        dst = qq[0:64,
                                                 gq * 1024 + h * 256:
                                                 gq * 1024 + (h + 1) * 256]
                                        rr_copy(
                                            dst,
                                            ppsum[hh * 64:(hh + 1) * 64,
                                                  qp * 256:(qp + 1) * 256])
                                        nc.vector.tensor_mul(
                                            sq[:, qp * 256:(qp + 1) * 256],
                                            dst, dst)
                                    nc.tensor.matmul(
                                        sspsum[:], ones_col[0:64, :], sq[:],
                                        start=(h == 0), stop=(h == 3))
                            else:
                                rr_copy(kh[0:64, tsl], ppsum[0:64, :])
                                sqk = wk.tile([64, TC], dtr, tag="sqk")
                                nc.vector.tensor_mul(
                                    sqk[:], kh[0:64, tsl], kh[0:64, tsl])
                                nc.tensor.matmul(
                                    sskp[:], ones_col[0:64, :], sqk[:],
                                    start=True, stop=True)
                                vtmp = wk.tile([64, TC], dt, tag="vt")
                                rr_copy(vtmp[:], ppsum[64:128, :])
                                for j in range(TC // 128):
                                    tp_ = ptr.tile([128, 64], dt, tag="tr")
                                    nc.tensor.transpose(
                                        tp_[:],
                                        vtmp[:, j * 128:(j + 1) * 128],
                                        ident[:])
                                    rr_copy(
                                        vaug[(ts0 + j * 128) // 128][:, 0:HD],
                                        tp_[:])
                        bsl = slice(ts0 - b * T, ts0 - b * T + TC)
                        ssq = wk.tile([1, TC], dt, tag="ssq")
                        nc.vector.tensor_copy(ssq[:], sspsum[:])
                        ssk = wk.tile([1, TC], dt, tag="ssk")
                        nc.vector.tensor_copy(ssk[:], sskp[:])
                        nc.sync.dma_start(cc_in[b][0:1, bsl], ssq[:])
                        nc.sync.dma_start(cc_in[b][1:2, bsl], ssk[:])
                    # per-batch AllReduce right after its sumsq is complete
                    nc.gpsimd.collective_compute(
                        "AllReduce", _mb.AluOpType.add,
                        replica_groups=[list(range(8))],
                        ins=[cc_in[b].opt()], outs=[cc_out[b].opt()])

            # ====== phase 2+3+4: rsqrt + normalize + attention per batch ======
            def rsqrt_batch(b):
                # rect [128, 32]: rows 0:64 q-ss, 64:128 k-ss; partition p
                # holds tokens 32p..32p+31 of batch b.
                ss_rect = wk.tile([128, 32], dt, tag="ssr", name="ssr")
                nc.sync.dma_start(
                    ss_rect[:],
                    cc_out[b][:].rearrange("r (p c) -> (r p) c", c=32))
                vaff = wk.tile([128, 32], dt, tag="vaff", name="vaff")
                nc.vector.tensor_mul(vaff[:], ss_rect[:], sc_col[:])
                nc.vector.tensor_add(vaff[:], vaff[:], bi_col[:])
                s1 = wk.tile([128, 32], dt, tag="s1", name="s1")
                nc.scalar.activation(s1[:], vaff[:], AF.Sqrt)
                y0 = wk.tile([128, 32], dt, tag="y0", name="y0")
                nc.vector.reciprocal(y0[:], s1[:])
                t1 = wk.tile([128, 32], dt, tag="t1", name="t1")
                nc.vector.tensor_mul(t1[:], y0[:], y0[:])
                nc.vector.tensor_mul(t1[:], t1[:], vaff[:])
                nc.scalar.activation(t1[:], t1[:], AF.Copy,
                                     bias=1.5, scale=-0.5)
                rs_fin = wk.tile([128, 32], dtr, tag="rsf", name="rsf")
                nc.vector.tensor_mul(rs_fin[:], y0[:], t1[:])
                nc.sync.dma_start(
                    rs_dram[b][:].rearrange("r (p c) -> (r p) c", c=32),
                    rs_fin[:])

            with (
                tc.tile_pool(name="ps_sc", bufs=3, space="PSUM") as psc,
                tc.tile_pool(name="ps_o", bufs=2, space="PSUM") as po,
                tc.tile_pool(name="ps_z", bufs=1, space="PSUM") as pz,
            ):
                def norm_batch(b):
                    for bc in range(T // TC):
                        ts0 = b * T + bc * TC
                        bsl = slice(bc * TC, (bc + 1) * TC)
                        rsq_t = wk.tile([1, TC], dtr, tag="rsq", name="rsq")
                        nc.sync.dma_start(rsq_t[:], rs_dram[b][0:1, bsl])
                        rsk_t = wk.tile([1, TC], dtr, tag="rsk", name="rsk")
                        nc.sync.dma_start(rsk_t[:], rs_dram[b][1:2, bsl])
                        gq0 = ts0 // 256
                        for h in range(QH):
                            zq = pz.tile([64, 2 * TQ], dt, tag="zb",
                                         name="zq")
                            nc.tensor.matmul(
                                zq[:], qnw_sb[0:1, h * 64:(h + 1) * 64],
                                rsq_t[:], start=True, stop=True)
                            for qp in range(2):
                                csl = slice((gq0 + qp) * 1024 + h * 256,
                                            (gq0 + qp) * 1024 +
                                            (h + 1) * 256)
                                nc.vector.tensor_mul(
                                    qq[0:64, csl], qq[0:64, csl],
                                    zq[0:64, qp * 256:(qp + 1) * 256])
                        zk = pz.tile([64, 2 * TQ], dt, tag="zb", name="zk")
                        nc.tensor.matmul(zk[0:64, 0:TC], knw_sb[:], rsk_t[:],
                                         start=True, stop=True)
                        nc.vector.tensor_mul(
                            kh[0:64, ts0:ts0 + TC], kh[0:64, ts0:ts0 + TC],
                            zk[0:64, 0:TC])

                def attn_batch(b):
                    for i in range(T // TQ):
                        t0 = i * TQ
                        g0 = b * T + t0
                        gq = g0 // 256
                        s0lo = max(0, t0 - WINDOW)
                        nblk = (t0 - s0lo) // 128 + 2
                        opair = [po.tile([65, 2 * TQ], dt, tag=f"op{p}",
                                         name=f"op{p}")
                                 for p in range(2)]
                        pv_pend = []
                        for j in range(nblk):
                            gs = b * T + s0lo + j * 128
                            scp = [psc.tile([128, 2 * TQ], dt, tag="sc",
                                            name="sc")
                                   for _ in range(2)]
                            for p in range(2):
                                nc.tensor.matmul(
                                    scp[p][:],
                                    kh[0:66, gs:gs + 128],
                                    qq[0:66, gq * 1024 + p * 512:
                                       gq * 1024 + (p + 1) * 512],
                                    start=True, stop=True)
                            if j == nblk - 2:
                                m = mask_sb[1]
                            elif j == nblk - 1:
                                m = mask_sb[2]
                            elif j == 0 and t0 >= WINDOW:
                                m = mask_sb[0]
                            elif j == 1 and t0 >= WINDOW:
                                m = mask_sb[3]
                            else:
                                m = None
                            pt = [ep.tile([128, 2 * TQ], bf, tag="e",
                                          name="e")
                                  for _ in range(2)]
                            for p in range(2):
                                if m is not None:
                                    nc.vector.tensor_add(
                                        scp[p][:], scp[p][:], m[:])
                                nc.scalar.activation(pt[p][:], scp[p][:],
                                                     AF.Exp)
                            if pv_pend:
                                pgs, ppt = pv_pend.pop()
                                for p in range(2):
                                    nc.tensor.matmul(
                                        opair[p][0:65, :],
                                        vaug[pgs // 128][:], ppt[p][:],
                                        start=(pgs == b * T + s0lo),
                                        stop=False)
                            pv_pend.append((gs, pt))
                        pgs, ppt = pv_pend.pop()
                        for p in range(2):
                            nc.tensor.matmul(
                                opair[p][0:65, :],
                                vaug[pgs // 128][:], ppt[p][:],
                                start=(pgs == b * T + s0lo), stop=True)
                        # softmax divide: rows 64 hold the denominators
                        for p in range(2):
                            zd = wk.tile([1, 2 * TQ], dt, tag="zd", bufs=1,
                                         name="zd")
                            nc.vector.tensor_copy(zd[:], opair[p][64:65, :])
                            zf = wk.tile([1, 2 * TQ], dt, tag="zf", bufs=1,
                                         name="zf")
                            nc.vector.reciprocal_approx_fast(zf[:], zd[:])
                            zr = wk.tile([1, 2 * TQ], dtr, tag="zr", bufs=1,
                                         name="zr")
                            nc.scalar.copy(zr[:], zf[:])
                            zbc = pz.tile([64, 2 * TQ], dt, tag="zb",
                                          name="zbc")
                            nc.tensor.matmul(zbc[:], ones_row[:], zr[:],
                                             start=True, stop=True)
                            zbs = wk.tile([64, 2 * TQ], dt, tag="zbs",
                                          bufs=1, name="zbs")
                            nc.scalar.copy(zbs[:], zbc[:])
                            # P-layout: cols = [h(2p) 256 | h(2p+1) 256]
                            for h2 in range(2):
                                nc.vector.tensor_mul(
                                    o2[p][h2 * 64:(h2 + 1) * 64,
                                          g0:g0 + TQ],
                                    opair[p][0:64, h2 * TQ:(h2 + 1) * TQ],
                                    zbs[:, h2 * TQ:(h2 + 1) * TQ])

                rsqrt_batch(0)
                norm_batch(0)
                attn_batch(0)
                rsqrt_batch(1)
                norm_batch(1)
                attn_batch(1)

            # ================= phase 5: wo projection =================
            with tc.tile_pool(name="ps_wo", bufs=8, space="PSUM") as pw:
                for ot in range(NKT):
                    osl = slice(ot * 128, (ot + 1) * 128)
                    for half in range(2):
                        wps = [(pw.tile([128, TC], dt, tag="wo", name="wp"),
                                slice((half * 4 + t_) * TC,
                                      (half * 4 + t_ + 1) * TC))
                               for t_ in range(4)]
                        for p in range(2):
                            for wp, csl in wps:
                                nc.tensor.matmul(
                                    wp[:], wo_sb[p][:, osl], o2[p][:, csl],
                                    start=(p == 0), stop=(p == 1))
                        for wp, csl in wps:
                            ost = op_.tile([128, TC], bf, tag="os",
                                           name="ost")
                            rr_copy(ost[:], wp[:])
                            nc.sync.dma_start(out_d[osl, csl], ost[:])
    nc.finalize()
    return nc


def _host_inputs(x, wq, wk, wv, wo, q_norm_w, k_norm_w):
    import ml_dtypes
    f32 = np.float32
    bf16 = ml_dtypes.bfloat16
    x = np.asarray(x, f32)
    xTb = np.ascontiguousarray(x.reshape(T4, DIM).T).astype(bf16)
    r = 2.0 ** (-8.0 / NH)
    slopes = np.asarray([r ** i for i in range(NH)], f32)
    sc_col = np.concatenate([np.full((64, 32), 1.0 / 32.0, f32),
                             np.full((64, 32), 1.0 / 512.0, f32)])
    bi_col = np.concatenate([np.full((64, 32), 64.0 * EPS, f32),
                             np.full((64, 32), EPS, f32)])
    # masks [4, 128, 512]: local key row sl, local query col ql (x2 heads).
    # 0: M_C oldest block (j=0, t0>=W):   mask sl <  ql
    # 1: M_A diag block  (j=nblk-2):      mask sl >  ql
    # 2: M_B diag block  (j=nblk-1):      mask sl+128 > ql
    # 3: M_D 2nd-oldest  (j=1, t0>=W):    mask sl+128 < ql
    sl = np.arange(128)[:, None]
    ql = np.arange(TQ)[None, :]
    NEG = f32(-1e30)
    m0 = np.where(sl < ql, NEG, 0.0).astype(f32)
    m1 = np.where(sl > ql, NEG, 0.0).astype(f32)
    m2 = np.where(sl + 128 > ql, NEG, 0.0).astype(f32)
    m3 = np.where(sl + 128 < ql, NEG, 0.0).astype(f32)
    masks = np.stack([np.concatenate([m, m], 1) for m in (m0, m1, m2, m3)])
    # positions centered at -1024: |slope*s~| <= 1024*slope keeps the fp22
    # rounding of the s-row within the bias-table error envelope; the
    # -slope*t~ row is constant per query column and cancels in softmax.
    tpos = (np.arange(T4) % T).astype(f32) - f32(WINDOW)
    in_maps = []
    for c in range(8):
        wTc = np.concatenate([
            np.asarray(wq, f32)[c * QD:(c + 1) * QD],
            np.asarray(wk, f32)[c * HD:(c + 1) * HD],
            np.asarray(wv, f32)[c * HD:(c + 1) * HD]], 0).T
        wTc = np.ascontiguousarray(wTc).astype(bf16)
        woTc = np.ascontiguousarray(
            np.asarray(wo, f32)[:, c * QD:(c + 1) * QD].T).astype(bf16)
        aug = np.stack([tpos, np.ones(T4, f32)])
        # aug2 in qq layout: col = gq*1024 + h*256 + ql
        aug2 = np.zeros((2, 4 * T4), f32)
        for gq in range(16):
            for h in range(QH):
                cs = gq * 1024 + h * 256
                tt = tpos[gq * 256:(gq + 1) * 256]
                aug2[0, cs:cs + 256] = slopes[4 * c + h]
                aug2[1, cs:cs + 256] = -slopes[4 * c + h] * tt
        in_maps.append({
            "xT": xTb,
            "wT": wTc,
            "woT": woTc,
            "qnw": np.asarray(q_norm_w, f32)[c * QD:(c + 1) * QD]
            .reshape(1, QD),
            "knw": np.asarray(k_norm_w, f32)[c * HD:(c + 1) * HD]
            .reshape(1, HD),
            "aug": aug,
            "aug2": aug2,
            "masks": masks,
            "ones_col": np.ones((128, 1), f32),
            "ones_row": np.ones((1, 64), f32),
            "ident": np.eye(64, dtype=f32),
            "sc_col": sc_col,
            "bi_col": bi_col,
        })
    return in_maps


def kernel(x, wq, wk, wv, wo, q_norm_w, k_norm_w):
    from concourse.bass_utils import run_bass_kernel_spmd
    if "nc" not in _CACHE:
        _CACHE["nc"] = _build_bass()
    nc = _CACHE["nc"]
    in_maps = _host_inputs(x, wq, wk, wv, wo, q_norm_w, k_norm_w)
    res = run_bass_kernel_spmd(nc, in_maps, core_ids=list(range(8)))
    out = np.zeros((DIM, T4), np.float64)
    for c in range(8):
        out += res.results[c]["out"].astype(np.float64)
    return np.ascontiguousarray(out.T).reshape(B, T, DIM).astype(np.float32)

